# revision 1
# baseline (speedup 1.0000x reference)
"""Trainium2 Bass kernel for time-varying all-pole (LPC) digital filter.

Reference computation (per batch sequence b):
    a_up = linear-interpolate frame coeffs (B,800,25) -> (B,64000,25)  (P=80)
    x~   = a_up[...,0] * x
    y[t] = x~[t] - sum_{m=1..24} a_up[t,m] * y[t-m]

Strategy:
  * Batch (32 seqs) data-parallel over 8 cores -> 4 seqs/core.
  * Each sequence is cut into 32 output windows of L=2000 samples. Each
    window is computed independently by running the exact recurrence from
    zero state starting W=160 samples early (overlap-discard). The filter
    is strongly stable (taps ~ N(0, 0.02), pole radius ~0.85-0.9), so the
    zero-state error decays below fp32 noise well within 160 samples.
  * 4 seqs x 32 windows = 128 independent windows -> one per SBUF
    partition. The recurrence runs in scatter form: when y[t] is final,
    one VectorE scalar_tensor_tensor op does
        ACC[t+1 : t+25] += y[t] * (-a_up[t+1..t+24, diag])
    so each sample costs a single [128, 24] DVE instruction. ACC is
    pre-filled with x~; after step t-1, ACC[t] IS y[t].
  * Coefficients: frame data (29 rows x 25 cols, taps pre-negated on
    host) is interpolated on-chip into a per-sample slab S[t, m] in
    4 chunks of 7 frames. The phi*delta product runs on ScalarE (80
    constant-scale ops per chunk, hidden under the DVE chain); the
    +frame-value add runs on DVE.

Self-contained: hardcodes all shapes; only imports the bass runtime.
"""

import sys

import numpy as np

sys.path.insert(0, "/opt/trn_rl_repo")

import concourse.bacc as bacc  # noqa: E402
import concourse.bass as bass  # noqa: E402
import concourse.mybir as mybir  # noqa: E402
import concourse.tile as tile  # noqa: E402
from concourse.bass_utils import run_bass_kernel_spmd  # noqa: E402

# Problem shapes
B, N, P, M = 32, 800, 80, 24
T = N * P  # 64000
NCORES = 8
SEQS = B // NCORES  # 4 seqs per core

# Windowing
L = 2000                 # output samples per window
W = 160                  # warmup samples (2 frames, keeps windows frame-aligned)
NS = W + L               # 2160 chain steps per window
NA = NS + M              # 2184 ACC slots (last 24 receive scatters, never read)
WPS = T // L             # 32 windows per sequence
NWIN = SEQS * WPS        # 128 windows per core = partitions

# Slab chunking (25 cols per sample: [gain, -a1..-a24])
NFR = 28                 # slab frames (28*80 = 2240 >= NA)
KF = NFR + 1             # frame rows per window (incl. next-frame row)
CF = 7                   # frames per chunk
NCHUNK = NFR // CF       # 4
CS = CF * P              # 560 chunk samples
CL = CS + M              # 584 local slab samples per chunk (24-sample header)

F32 = mybir.dt.float32
MULT = mybir.AluOpType.mult
ADD = mybir.AluOpType.add
SUB = mybir.AluOpType.subtract


def _sv(t_ap, off, pairs):
    """Strided free-dim view of a [128, F] tile AP: same tensor/partitions,
    custom free access pattern (list of [step, count])."""
    row = t_ap.ap[0][0]
    return bass.AP(t_ap.tensor, t_ap.offset + off, [[row, 128]] + pairs)


def _build_program(compile=True):
    nc = bacc.Bacc("TRN2", target_bir_lowering=False, debug=False)

    xw_d = nc.dram_tensor("xw", [NWIN, NS], F32, kind="ExternalInput")
    fr_d = nc.dram_tensor("fr", [NWIN, KF * 25], F32, kind="ExternalInput")
    phi_d = nc.dram_tensor("phi", [NWIN, P], F32, kind="ExternalInput")
    y_d = nc.dram_tensor("y", [NWIN, L], F32, kind="ExternalOutput")

    with tile.TileContext(nc) as tc:
        with (
            tc.tile_pool(name="const", bufs=1) as cpool,
            tc.tile_pool(name="slab", bufs=2) as spool,
        ):
            XW = cpool.tile([128, NS], F32, tag="xw")
            FR = cpool.tile([128, KF * 25], F32, tag="fr")
            PHI = cpool.tile([128, P], F32, tag="phi")
            D = cpool.tile([128, NFR * 25], F32, tag="d")
            KG = cpool.tile([128, NS], F32, tag="kg")
            ACC = cpool.tile([128, NA], F32, tag="acc")

            nc.sync.dma_start(XW[:], xw_d.ap())
            nc.sync.dma_start(FR[:], fr_d.ap())
            nc.sync.dma_start(PHI[:], phi_d.ap())

            # D[k] = FR[k+1] - FR[k]   (28 frame rows of 25)
            nc.vector.tensor_tensor(
                D[:], FR[:, 25 : KF * 25], FR[:, 0 : NFR * 25], SUB
            )

            # ---- gain lerp for the whole window: KG[t] = FRg[k] + phi*Dg[k]
            # t = k*80 + p, k in [0,27), p in [0,80)
            for p in range(P):
                # KG[:, k*80+p for k in 0..26] = D[k*25] * (p/80)   (ScalarE)
                nc.scalar.mul(
                    _sv(KG[:], p, [[P, NS // P]]),
                    _sv(D[:], 0, [[25, NS // P]]),
                    float(p) / P,
                )
            # KG += FR gain col broadcast over phase
            nc.vector.tensor_tensor(
                _sv(KG[:], 0, [[P, NS // P], [1, P]]),
                _sv(FR[:], 0, [[25, NS // P], [0, P]]),
                _sv(KG[:], 0, [[P, NS // P], [1, P]]),
                ADD,
            )
            # ACC[0:NS] = KG * XW ; ACC tail zeroed
            nc.vector.tensor_tensor(ACC[:, 0:NS], KG[:], XW[:], MULT)
            nc.vector.memset(ACC[:, NS:NA], 0.0)

            # ---- chunks
            for c in range(NCHUNK):
                S = spool.tile([128, CL * 25], F32, tag="s")
                k0 = c * CF  # first main frame of this chunk
                # header: slab samples [560c-24, 560c) = phases 56..79 of
                # frame k0-1 (skip for c=0: those samples are never read)
                if c > 0:
                    for p in range(P - M, P):
                        nc.scalar.mul(
                            _sv(S[:], (p - (P - M)) * 25, [[1, 25]]),
                            _sv(D[:], (k0 - 1) * 25, [[1, 25]]),
                            float(p) / P,
                        )
                    nc.vector.tensor_tensor(
                        _sv(S[:], 0, [[1, M * 25]]),
                        _sv(FR[:], (k0 - 1) * 25, [[0, M], [1, 25]]),
                        _sv(S[:], 0, [[1, M * 25]]),
                        ADD,
                    )
                # main: slab samples [560c, 560c+560), frames k0..k0+6
                for p in range(P):
                    nc.scalar.mul(
                        _sv(S[:], (M + p) * 25, [[P * 25, CF], [1, 25]]),
                        _sv(D[:], k0 * 25, [[25, CF], [1, 25]]),
                        float(p) / P,
                    )
                nc.vector.tensor_tensor(
                    _sv(S[:], M * 25, [[1, CS * 25]]),
                    _sv(FR[:], k0 * 25, [[25, CF], [0, P], [1, 25]]),
                    _sv(S[:], M * 25, [[1, CS * 25]]),
                    ADD,
                )

                # ---- the chain: steps t in [560c-24, 560c+536) clipped
                s_lo = max(0, c * CS - M)
                s_hi = min(NS, c * CS + CS - M)
                base = c * CS - M  # slab-local sample 0 == window sample base
                for t in range(s_lo, s_hi):
                    ell = t - base
                    nc.vector.scalar_tensor_tensor(
                        ACC[:, t + 1 : t + 25],
                        _sv(S[:], ell * 25 + 26, [[26, M]]),
                        ACC[:, t : t + 1],
                        ACC[:, t + 1 : t + 25],
                        MULT,
                        ADD,
                    )

                # output: positions [max(W, s_lo), s_hi) are final
                r0 = max(W, s_lo)
                nc.sync.dma_start(
                    y_d.ap()[:, r0 - W : s_hi - W], ACC[:, r0:s_hi]
                )

    if compile:
        nc.compile()
    return nc


_NC = None


def _host_prep(x, a):
    x = np.ascontiguousarray(x, np.float32)
    a = np.ascontiguousarray(a, np.float32)
    # extended frames: 2 zero rows in front (window 0 starts at frame -2),
    # last row replicated (interp clamps at the end). taps negated.
    a_ext = np.zeros((B, N + 5, 25), np.float32)
    a_ext[:, 2 : 2 + N] = a
    a_ext[:, 2 + N :] = a[:, -1:]
    a_ext[:, :, 1:] *= -1.0

    xpad = np.concatenate([np.zeros((B, W), np.float32), x], axis=1)
    xw = np.stack(
        [xpad[:, w * L : w * L + NS] for w in range(WPS)], axis=1
    )  # (B, WPS, NS)
    fr = np.stack(
        [a_ext[:, 25 * w : 25 * w + KF] for w in range(WPS)], axis=1
    )  # (B, WPS, KF, 25)

    phi = np.tile((np.arange(P, dtype=np.float32) / P), (128, 1))

    in_maps = []
    for c in range(NCORES):
        sl = slice(c * SEQS, (c + 1) * SEQS)
        in_maps.append(
            {
                "xw": np.ascontiguousarray(xw[sl].reshape(NWIN, NS)),
                "fr": np.ascontiguousarray(fr[sl].reshape(NWIN, KF * 25)),
                "phi": phi,
            }
        )
    return in_maps


def kernel(x, a, _trace=False, _trace_kwargs=None):
    global _NC
    if _NC is None:
        _NC = _build_program()

    in_maps = _host_prep(x, a)
    kw = {}
    if _trace:
        kw = dict(trace=True, trace_cores=[0], **(_trace_kwargs or {}))
    res = run_bass_kernel_spmd(_NC, in_maps, core_ids=list(range(NCORES)), **kw)

    y = np.empty((B, T), np.float32)
    for c in range(NCORES):
        yw = res.results[c]["y"].reshape(SEQS, WPS, L)
        y[c * SEQS : (c + 1) * SEQS] = yw.reshape(SEQS, T)
    kernel.last_results = res
    return y

